# revision 8
# baseline (speedup 1.0000x reference)
"""Fused transformer block (LN1 -> 16-head causal attention -> LN2 -> FFN,
two residuals) on 8 Trainium2 NeuronCores.

Sharding strategy
-----------------
- Attention is head-parallel: core c owns heads (2c, 2c+1) and computes their
  Q^T/K^T/V^T and attention output O^T for ALL 4096 tokens (B*T flattened).
- LN1 + transpose of the normalized activations is sequence-parallel (512
  tokens per core), stitched with an AllGather of x^T (feature-major).
- An AllToAll converts the head-parallel attention output into token-parallel
  layout; residual + LN2 + the whole FFN then run sequence-parallel (512
  tokens per core) with zero further communication. Each core returns its
  512-token slice of the output; the host concatenates.

Performance notes:
- LN gains/biases are folded into the weights host-side: Wq/Wk/Wv absorb
  ln1_g (per-head bias vectors absorb ln1_b), W1 absorbs ln2_g and b1
  absorbs ln2_b@W1. LN on device is just (x - mean) * rstd.
- A tiny dummy AllGather fires at kernel start to absorb the ~10us ncfw
  first-collective warmup, so the real AllGather begins ~0.5us after its
  trigger.
- The x^T AllGather is split into two feature-halves; QKV accumulation is
  split k0-3 / k4-7 so the second half of the collective overlaps the first
  half's matmuls (2 chunks in flight, 6 PSUM banks).
- wq/wk/wv travel as contiguous [128, 1024] rows (one 2KB descriptor per
  partition) so the DMA queues clear early and the emb tiles land fast.
- emb is loaded once; the same SBUF tiles feed LN1 and the phase-H residual.
- Causal masking is an in-place affine_select on the (otherwise idle) GpSimd
  engine, replacing per-tile Vector multiplies and the mask tiles.
- W1 is preloaded at the start of attention (DMA queues are idle there) so
  the 8 MB of reads never contend with the AllToAll.
- Attention is software-pipelined across chunks: scores+exp of chunk c+1 are
  emitted before the PV accumulation of chunk c, so the PE always has
  runnable work while the Scalar engine drains the softmax exps. Two score
  tiles share a 2-bank PSUM tile so one Exp covers 1024 columns.
- The attention-output AllToAll travels in bf16 (halves the exchange).
"""

import sys

if "/opt/trn_rl_repo" not in sys.path:
    sys.path.insert(0, "/opt/trn_rl_repo")

import ml_dtypes
import numpy as np

import concourse.bass as bass
import concourse.mybir as mybir
import concourse.tile as tile
from concourse import bacc
from concourse.bass import ds, ts
from concourse.masks import make_identity

# ── Problem constants (hardcoded; see spec) ──────────────────────────────────
B, T, D = 2, 2048, 1024
H, HS = 16, 64
FF = 4 * D  # 4096
EPS = 1e-5
NCORES = 8
NT = B * T  # 4096 tokens
TC = NT // NCORES  # 512 tokens per core
NTT = TC // 128  # 4 token tiles per core
HPC = H // NCORES  # 2 heads per core
SCALE = 1.0 / float(np.sqrt(D))  # reference scales scores by D**-0.5

F32 = mybir.dt.float32
BF16 = mybir.dt.bfloat16
AF = mybir.ActivationFunctionType
OP = mybir.AluOpType


def build():
    nc = bacc.Bacc(num_devices=NCORES)

    emb = nc.dram_tensor("emb", [TC, D], F32, kind="ExternalInput")
    # per-core head slice of Wq/Wk/Wv (ln1_g folded in), contiguous
    # [128(d_in), 1024(8 d_tiles x 2*HS)] rows -> one 2KB descriptor per row
    wq = nc.dram_tensor("wq", [128, 1024], BF16, kind="ExternalInput")
    wk = nc.dram_tensor("wk", [128, 1024], BF16, kind="ExternalInput")
    wv = nc.dram_tensor("wv", [128, 1024], BF16, kind="ExternalInput")
    # per-core QKV bias columns (ln1_b folded through the projections)
    bqkv = nc.dram_tensor("bqkv", [128, 3], F32, kind="ExternalInput")
    w1 = nc.dram_tensor("w1", [D, FF], BF16, kind="ExternalInput")
    w2 = nc.dram_tensor("w2", [FF, D], BF16, kind="ExternalInput")
    b1r = nc.dram_tensor("b1r", [128, FF // 128], F32, kind="ExternalInput")
    b2f = nc.dram_tensor("b2f", [D], F32, kind="ExternalInput")
    out = nc.dram_tensor("out", [TC, D], F32, kind="ExternalOutput")

    rg = [list(range(NCORES))]

    with tile.TileContext(nc) as tc:
        # Pools are allocated just-in-time and released LIFO per (space, side).
        const = tc.alloc_tile_pool(name="const", bufs=1)
        dram = tc.alloc_tile_pool(name="dram", bufs=1, space="DRAM")
        stat = tc.alloc_tile_pool(name="stat", bufs=4)
        xln = tc.alloc_tile_pool(name="xln", bufs=2)
        embs = tc.alloc_tile_pool(name="embs", bufs=1, side="right")

        # DRAM bounce buffers
        dummy_in = dram.tile([1, 64], BF16, name="dummy_in")
        dummy_out = dram.tile([NCORES, 64], BF16, name="dummy_out", addr_space="Shared")
        cc_x1 = dram.tile([D // 2, TC], BF16, name="cc_x1")
        cc_x2 = dram.tile([D // 2, TC], BF16, name="cc_x2")
        xg1 = dram.tile([NCORES * (D // 2), TC], BF16, name="xg1", addr_space="Shared")
        xg2 = dram.tile([NCORES * (D // 2), TC], BF16, name="xg2", addr_space="Shared")
        cc_a_in = dram.tile([NCORES, 130, TC], BF16, name="cc_a_in")
        cc_a_out = dram.tile([NCORES, 130, TC], BF16, name="cc_a_out")

        # Warmup collective: absorbs the ~10us ncfw first-trigger latency so
        # the real AllGather below starts almost immediately after its trigger.
        nc.gpsimd.collective_compute(
            "AllGather", OP.bypass, replica_groups=rg,
            ins=[dummy_in.opt()], outs=[dummy_out.opt()],
        )

        identity = const.tile([128, 128], F32, name="identity")
        make_identity(nc, identity)
        identity_b = const.tile([128, 128], BF16, name="identity_b")
        nc.vector.tensor_copy(out=identity_b, in_=identity)
        eps_t = const.tile([128, 1], F32, name="eps_t")
        nc.vector.memset(eps_t, EPS)
        b1s = const.tile([128, FF // 128], F32, name="b1s")
        nc.sync.dma_start(out=b1s, in_=b1r[:, :])
        b2b = const.tile([128, D], F32, name="b2b")
        b2a = b2f[:]
        nc.sync.dma_start(
            out=b2b,
            in_=bass.AP(tensor=b2a.tensor, offset=b2a.offset, ap=[[0, 128], [1, D]]),
        )
        bqs = const.tile([128, 3], F32, name="bqs")
        nc.sync.dma_start(out=bqs, in_=bqkv[:, :])

        # attention-lifetime pools (released after the AllToAll)
        qkres = tc.alloc_tile_pool(name="qkres", bufs=1)
        vsbp = tc.alloc_tile_pool(name="vsbp", bufs=1)
        otp = tc.alloc_tile_pool(name="otp", bufs=2)
        ptp = tc.alloc_tile_pool(name="ptp", bufs=6)
        attnc = tc.alloc_tile_pool(name="attnc", bufs=1)

        qT = qkres.tile([128, NT], BF16, name="qT")
        kT = qkres.tile([128, NT], BF16, name="kT")
        v_sb = vsbp.tile([128, NT // 128, HPC, 65], BF16, name="v_sb")
        ones_f = attnc.tile([128, 64], F32, name="ones_f")
        nc.vector.memset(ones_f, 1.0)

        # QKV-lifetime pools
        vtp = tc.alloc_tile_pool(name="vtp", bufs=1)
        wqkvc = tc.alloc_tile_pool(name="wqkvc", bufs=1)
        xrhs = tc.alloc_tile_pool(name="xrhs", bufs=2)
        vT = vtp.tile([128, NT], BF16, name="vT")
        wqs = wqkvc.tile([128, 1024], BF16, name="wqs")
        nc.sync.dma_start(out=wqs, in_=wq[:, :])
        wks = wqkvc.tile([128, 1024], BF16, name="wks")
        nc.sync.dma_start(out=wks, in_=wk[:, :])
        wvs = wqkvc.tile([128, 1024], BF16, name="wvs")
        nc.sync.dma_start(out=wvs, in_=wv[:, :])
        # W1 row-blocks 0-4 preload during attention (right-side stack, so the
        # late release keeps LIFO order); blocks 5-7 load in phase H.
        w1pa = tc.alloc_tile_pool(name="w1pa", bufs=1, side="right")

        def layer_norm(src_tile, dst_tile, use_scalar=False):
            """dst = (src - mean) * rstd  (per 128-token tile, stats over D).

            LN gain/bias are folded into the downstream weights host-side.
            With use_scalar the wide normalize pass runs on the Scalar engine
            as Identity(src * rstd + (-mean * rstd)), halving the Vector load.
            """
            st = stat.tile([128, 2, 6], F32, name="st", tag="st")
            nc.vector.bn_stats(out=st[:, 0, :], in_=src_tile[:, 0:512])
            nc.vector.bn_stats(out=st[:, 1, :], in_=src_tile[:, 512:1024])
            mv = stat.tile([128, 2], F32, name="mv", tag="mv")
            nc.vector.bn_aggr(out=mv, in_=st)
            std = stat.tile([128, 1], F32, name="std", tag="std")
            nc.scalar.activation(
                out=std, in_=mv[:, 1:2], func=AF.Sqrt, bias=eps_t, scale=1.0
            )
            rstd = stat.tile([128, 1], F32, name="rstd", tag="rstd")
            nc.vector.reciprocal(out=rstd, in_=std)
            if use_scalar:
                nm = stat.tile([128, 1], F32, name="nm", tag="nm")
                nc.vector.tensor_scalar(
                    out=nm,
                    in0=mv[:, 0:1],
                    scalar1=rstd,
                    scalar2=-1.0,
                    op0=OP.mult,
                    op1=OP.mult,
                )
                nc.scalar.activation(
                    out=dst_tile, in_=src_tile, func=AF.Identity, bias=nm, scale=rstd
                )
            else:
                nc.vector.tensor_scalar(
                    out=dst_tile,
                    in0=src_tile,
                    scalar1=mv[:, 0:1],
                    scalar2=rstd,
                    op0=OP.subtract,
                    op1=OP.mult,
                )

        # ── Phase A+B: LN1 on own 512-token chunk, transpose, send to AGs ──
        # emb tiles are persistent: they feed LN1 here and the attn residual
        # in phase H (loaded once).
        xtp = tc.alloc_tile_pool(name="xtp", bufs=1)
        ab_tp = tc.alloc_tile_pool(name="ab_tp", bufs=4, space="PSUM")
        xt_tiles = [xtp.tile([128, TC], BF16, name=f"xt{k}") for k in range(8)]
        et_tiles = []
        for i in range(NTT):
            et = embs.tile([128, D], F32, name="et", tag=f"et{i}", bufs=1)
            nc.sync.dma_start(out=et, in_=emb[ts(i, 128), :])
            et_tiles.append(et)
        xn_tiles = []
        for i in range(NTT):
            xn = xln.tile([128, D], F32, name="xn", tag="xn", bufs=4)
            layer_norm(et_tiles[i], xn, use_scalar=(i % 2 == 1))
            xn_tiles.append(xn)
        # k-major transposes with a per-block DMA: each feature block of the
        # AllGather input ships as soon as it is complete; the first half
        # (k=0..3) triggers AG#1 while k=4..7 are still transposing.
        for k in range(8):
            for i in range(NTT):
                ps = ab_tp.tile([128, 128], F32, name="abtp", tag="abtp")
                nc.tensor.transpose(ps, xn_tiles[i][:, ts(k, 128)], identity)
                if k % 2 == 0:
                    nc.vector.tensor_copy(out=xt_tiles[k][:, ts(i, 128)], in_=ps)
                else:
                    nc.scalar.copy(out=xt_tiles[k][:, ts(i, 128)], in_=ps)
            if k < 4:
                nc.sync.dma_start(out=cc_x1[ts(k, 128), :], in_=xt_tiles[k])
            else:
                nc.sync.dma_start(out=cc_x2[ts(k - 4, 128), :], in_=xt_tiles[k])
            if k == 3:
                # ── Phase C1: AllGather first feature half ──
                nc.gpsimd.collective_compute(
                    "AllGather", OP.bypass, replica_groups=rg,
                    ins=[cc_x1.opt()], outs=[xg1.opt()],
                )
        # ── Phase C2: AllGather second feature half ──
        nc.gpsimd.collective_compute(
            "AllGather", OP.bypass, replica_groups=rg,
            ins=[cc_x2.opt()], outs=[xg2.opt()],
        )
        xtp.release()
        ab_tp.release()

        # ── Phase D: Q^T/K^T/V^T for own 2 heads over all 4096 tokens ──
        # Split accumulation: k=0..3 from AG#1, k=4..7 from AG#2, with two
        # chunks in flight (6 PSUM banks) so AG#2's transfer overlaps the
        # first chunks' matmuls.
        qkv_ps = tc.alloc_tile_pool(name="qkv_ps", bufs=2, space="PSUM")

        def dma_x(cb, half, xg):
            xr = xrhs.tile([128, 4, TC], BF16, name="xr", tag=f"xr{half}", bufs=2)
            xga = xg[:]
            src = bass.AP(
                tensor=xga.tensor,
                offset=xga.offset + (D // 2) * cb * TC,
                ap=[[TC, 128], [128 * TC, 4], [1, TC]],
            )
            nc.sync.dma_start(out=xr, in_=src)
            return xr

        def emit_lo(cb):
            xr = dma_x(cb, 0, xg1)
            pq = qkv_ps.tile([128, TC], F32, name="pq", tag="pq")
            pk = qkv_ps.tile([128, TC], F32, name="pk", tag="pk")
            pv = qkv_ps.tile([128, TC], F32, name="pv", tag="pv")
            for k in range(4):
                kw = dict(start=(k == 0), stop=False)
                nc.tensor.matmul(pq, lhsT=wqs[:, ts(k, 128)], rhs=xr[:, k, :], **kw)
                nc.tensor.matmul(pk, lhsT=wks[:, ts(k, 128)], rhs=xr[:, k, :], **kw)
                nc.tensor.matmul(pv, lhsT=wvs[:, ts(k, 128)], rhs=xr[:, k, :], **kw)
            return (pq, pk, pv)

        def emit_hi(cb, psums):
            pq, pk, pv = psums
            xr = dma_x(cb, 1, xg2)
            for k in range(4):
                kw = dict(start=False, stop=(k == 3))
                kk = k + 4
                nc.tensor.matmul(pq, lhsT=wqs[:, ts(kk, 128)], rhs=xr[:, k, :], **kw)
                nc.tensor.matmul(pk, lhsT=wks[:, ts(kk, 128)], rhs=xr[:, k, :], **kw)
                nc.tensor.matmul(pv, lhsT=wvs[:, ts(kk, 128)], rhs=xr[:, k, :], **kw)
            nc.vector.tensor_scalar_add(
                out=qT[:, ts(cb, TC)], in0=pq, scalar1=bqs[:, 0:1]
            )
            nc.vector.tensor_scalar_add(
                out=kT[:, ts(cb, TC)], in0=pk, scalar1=bqs[:, 1:2]
            )
            nc.scalar.activation(
                out=vT[:, ts(cb, TC)], in_=pv, func=AF.Identity,
                bias=bqs[:, 2:3], scale=1.0,
            )

        prev_qkv = None
        for cb in range(NCORES):
            psums = emit_lo(cb)
            if prev_qkv is not None:
                emit_hi(prev_qkv[0], prev_qkv[1])
            prev_qkv = (cb, psums)
        emit_hi(prev_qkv[0], prev_qkv[1])

        # ── Phase E: V^T -> natural V per s-tile, ones-augmented ────────
        for st_ in range(NT // 128):
            ps = qkv_ps.tile([128, 128], BF16, name="tpb", tag="tpb")
            nc.tensor.transpose(ps, vT[:, ts(st_, 128)], identity_b)
            nc.vector.tensor_copy(
                out=v_sb[:, st_, :, 0:64],
                in_=ps.rearrange("p (h e) -> p h e", h=HPC),
            )
        nc.vector.tensor_copy(
            out=v_sb[:, :, :, 64:65],
            in_=ones_f[:, 0:64].rearrange("p (a b c) -> p a b c", a=NT // 128, b=HPC),
        )
        xrhs.release()
        wqkvc.release()
        vtp.release()
        qkv_ps.release()

        # ── Phase F: causal attention, software-pipelined across chunks ──
        # Per chunk: S^T matmuls stream through PSUM banks, exp evacuates to
        # bf16 SBUF; causal masking is an in-place affine_select on GpSimd.
        # The scores+exp of chunk c+1 are emitted BEFORE the PV accumulation
        # of chunk c so the PE has dependency-free work while Scalar drains.
        s_ps = tc.alloc_tile_pool(name="s_ps", bufs=3, space="PSUM")
        o_ps = tc.alloc_tile_pool(name="o_ps", bufs=1, space="PSUM")

        # W1 partial preload: DMA queues are idle during attention, so 5 MB of
        # the reads finish long before the AllToAll needs the wires.
        w1sb = [w1pa.tile([128, FF], BF16, name=f"w1sb{k}") for k in range(5)]
        for k in range(5):
            nc.sync.dma_start(out=w1sb[k], in_=w1[ts(k, 128), :])

        def emit_scores(gc):
            b = gc // 4
            lc = gc % 4
            nst = 4 * lc + 4  # s-tiles (128 wide) within this batch
            t0g = gc * TC
            pts = {}
            # two s-tiles share one 2-bank PSUM tile so a single Exp covers
            # 1024 columns (halves the Scalar per-instruction overhead)
            # Causal column-skipping: a diag-d tile's first 128*d query
            # columns never survive the mask, so every stage (scores, exp,
            # mask, PV) is restricted to the columns the next stage reads.
            for sp in range(nst // 2):
                d0 = 2 * sp - 4 * lc
                d1 = d0 + 1
                cl = 128 * d0 if d0 > 0 else 0  # pair-wide exp start column
                for h in range(HPC):
                    ps_ = s_ps.tile([128, 2, TC], F32, name="ps_", tag="ps_")
                    pt_ = ptp.tile([128, 2, TC], BF16, name="pt_", tag="pt_", bufs=29)
                    for hf in range(2):
                        stl = 2 * sp + hf
                        sg = b * 16 + stl
                        diag = stl - 4 * lc
                        c0 = 128 * diag if diag > 0 else 0
                        # heads use PE row-groups 0-63 / 64-127 -> concurrent
                        nc.tensor.matmul(
                            ps_[:, hf, ds(c0, TC - c0)],
                            lhsT=kT[ts(h, 64), ts(sg, 128)],
                            rhs=qT[ts(h, 64), ds(t0g + c0, TC - c0)],
                            start=True,
                            stop=True,
                        )
                    nc.scalar.activation(
                        out=pt_[:, :, ds(cl, TC - cl)],
                        in_=ps_[:, :, ds(cl, TC - cl)],
                        func=AF.Exp,
                        scale=SCALE,
                    )
                    if d1 >= 0:
                        # causal triangle of the diagonal pair, zeroed in
                        # place on GpSimd: keep where col >= 128*diag + row
                        nc.gpsimd.affine_select(
                            out=pt_[:, :, ds(cl, TC - cl)],
                            in_=pt_[:, :, ds(cl, TC - cl)],
                            pattern=[[-128, 2], [1, TC - cl]],
                            compare_op=OP.is_ge,
                            fill=0.0,
                            base=cl - 128 * d0,
                            channel_multiplier=-1,
                        )
                    for hf in range(2):
                        stl = 2 * sp + hf
                        diag = stl - 4 * lc
                        c0 = 128 * diag if diag > 0 else 0
                        pts[(stl, h)] = (pt_, hf, c0)
            return pts

        def emit_pv(gc, pts):
            b = gc // 4
            lc = gc % 4
            nst = 4 * lc + 4
            oT = otp.tile([128, TC], BF16, name="oT", tag="oT")
            for h in range(HPC):
                po = o_ps.tile([65, TC], F32, name=f"po{h}", tag=f"po{h}", bufs=1)
                for stl in range(nst):
                    sg = b * 16 + stl
                    pt_, hf, c0 = pts[(stl, h)]
                    nc.tensor.matmul(
                        po[:, ds(c0, TC - c0)],
                        lhsT=v_sb[:, sg, h, :],
                        rhs=pt_[:, hf, ds(c0, TC - c0)],
                        start=(stl == 0),
                        stop=(stl == nst - 1),
                        skip_group_check=(c0 > 0),
                    )
                nc.vector.tensor_copy(out=oT[ts(h, 64), :], in_=po[0:64, :])
                oTd = otp.tile([1, TC], BF16, name="oTd", tag="oTd")
                nc.vector.tensor_copy(out=oTd, in_=po[64:65, :])
                nc.sync.dma_start(out=cc_a_in[gc, 128 + h : 129 + h, :], in_=oTd)
            nc.sync.dma_start(out=cc_a_in[gc, 0:128, :], in_=oT)

        prev = None
        for gc in range(NCORES):
            pts = emit_scores(gc)
            if prev is not None:
                emit_pv(prev[0], prev[1])
            prev = (gc, pts)
        emit_pv(prev[0], prev[1])

        # ── Phase G: AllToAll -> unnormalized attn^T + denoms, own tokens ──
        nc.gpsimd.collective_compute(
            "AllToAll", OP.bypass, replica_groups=rg, ins=[cc_a_in.opt()], outs=[cc_a_out.opt()]
        )
        o_ps.release()
        s_ps.release()
        attnc.release()
        ptp.release()
        otp.release()
        vsbp.release()
        qkres.release()

        # ── Phase H: normalize + attn residual + LN2, y -> y^T ──────────
        asbp = tc.alloc_tile_pool(name="asbp", bufs=4, side="right")
        h_tp = tc.alloc_tile_pool(name="h_tp", bufs=5, space="PSUM")
        x2p = tc.alloc_tile_pool(name="x2p", bufs=1)
        ytp = tc.alloc_tile_pool(name="ytp", bufs=1)
        w1pb = tc.alloc_tile_pool(name="w1pb", bufs=1)
        x2_tiles = [x2p.tile([128, D], F32, name=f"x2_{i}") for i in range(NTT)]
        yt_tiles = [ytp.tile([128, TC], BF16, name=f"yt{k}") for k in range(8)]
        asb_tiles = []
        for c in range(NCORES):
            asb = asbp.tile([128, TC], BF16, name="asb", tag="asb", bufs=8)
            nc.sync.dma_start(out=asb, in_=cc_a_out[c, 0:128, :])
            dnm = asbp.tile([2, TC], BF16, name="dnm", tag="dnm", bufs=8)
            nc.sync.dma_start(out=dnm, in_=cc_a_out[c, 128:130, :])
            asb_tiles.append((asb, dnm))
        # W1 row-blocks 5-7: queued behind the asb DMAs, so they drain right
        # after the AllToAll completes instead of contending with it.
        for k in range(5, 8):
            w1sb.append(w1pb.tile([128, FF], BF16, name=f"w1sb{k}"))
            nc.sync.dma_start(out=w1sb[k], in_=w1[ts(k, 128), :])
        # i-major: each x2 tile completes as early as possible so its LN2
        # (emitted right after) overlaps the remaining residual work
        yn_tiles = []
        for i in range(NTT):
            for c in range(NCORES):
                asb, dnm = asb_tiles[c]
                pn = h_tp.tile([128, 128], BF16, name="htp", tag="htp")
                nc.tensor.transpose(pn, asb[:, ts(i, 128)], identity_b)
                pd = h_tp.tile([128, 2], BF16, name="hpd", tag="htp")
                nc.tensor.transpose(pd, dnm[:, ts(i, 128)], identity_b[0:2, 0:2])
                rcp = asbp.tile([128, 2], F32, name="rcp", tag="rcp")
                nc.vector.reciprocal(out=rcp, in_=pd)
                for h in range(HPC):
                    nc.vector.scalar_tensor_tensor(
                        out=x2_tiles[i][:, ds(128 * c + 64 * h, 64)],
                        in0=pn[:, ts(h, 64)],
                        scalar=rcp[:, h : h + 1],
                        in1=et_tiles[i][:, ds(128 * c + 64 * h, 64)],
                        op0=OP.mult,
                        op1=OP.add,
                    )
            # shares the xn tag: the phase-A xn tiles are dead by now, and
            # reusing their buffers keeps the SBUF budget under the limit
            yn = xln.tile([128, D], F32, name="yn", tag="xn", bufs=4)
            layer_norm(x2_tiles[i], yn, use_scalar=(i % 2 == 1))
            yn_tiles.append(yn)
        # k-major transposes: yt[0] (which gates the FFN's first matmul)
        # completes first instead of last
        for k in range(8):
            for i in range(NTT):
                ps = h_tp.tile([128, 128], F32, name="htp2", tag="htp2", bufs=3)
                nc.tensor.transpose(ps, yn_tiles[i][:, ts(k, 128)], identity)
                if k % 2 == 0:
                    nc.vector.tensor_copy(out=yt_tiles[k][:, ts(i, 128)], in_=ps)
                else:
                    nc.scalar.copy(out=yt_tiles[k][:, ts(i, 128)], in_=ps)
        asbp.release()
        h_tp.release()

        # ── Phase J: FFN up-projection, h^T = relu(W1^T y^T + b1) ───────
        htp = tc.alloc_tile_pool(name="htp", bufs=1)
        w2sp = tc.alloc_tile_pool(name="w2sp", bufs=6)
        outsp = tc.alloc_tile_pool(name="outs", bufs=1)
        h_ps = tc.alloc_tile_pool(name="h_ps", bufs=4, space="PSUM")
        ht_tiles = [htp.tile([128, TC], BF16, name=f"ht{j}") for j in range(FF // 128)]
        out_sb = [outsp.tile([128, D], F32, name=f"osb{i}") for i in range(NTT)]
        # fold the down-projection bias into the residual while Vector is idle
        # (LN2 has already consumed x2, so this is safe)
        for i in range(NTT):
            nc.vector.tensor_add(out=x2_tiles[i], in0=x2_tiles[i], in1=b2b)
        for jg in range(16):
            phs = [h_ps.tile([128, TC], F32, name=f"ph{jj}", tag="ph") for jj in range(2)]
            for k in range(8):
                for jj in range(2):
                    nc.tensor.matmul(
                        phs[jj],
                        lhsT=w1sb[k][:, ds(256 * jg + 128 * jj, 128)],
                        rhs=yt_tiles[k],
                        start=(k == 0),
                        stop=(k == 7),
                    )
            for jj in range(2):
                jt = 2 * jg + jj
                nc.scalar.activation(
                    out=ht_tiles[jt],
                    in_=phs[jj],
                    func=AF.Relu,
                    bias=b1s[:, jt : jt + 1],
                    scale=1.0,
                )
        h_ps.release()

        # ── Phase K: FFN down-projection, natural [token, D] accumulation ──
        # lhsT is an h^T chunk reused for both 512-wide halves of W2's rows;
        # each token tile owns a 2-bank PSUM accumulator, so the output needs
        # no final transposes — just one residual add per tile.
        f_ps = tc.alloc_tile_pool(name="f_ps", bufs=4, space="PSUM")
        pfs = [f_ps.tile([128, D], F32, name=f"pf{i}", tag="pf") for i in range(NTT)]
        for jt in range(FF // 128):
            w2t = w2sp.tile([128, D], BF16, name="w2t", tag="w2t")
            nc.sync.dma_start(out=w2t, in_=w2[ts(jt, 128), :])
            for i in range(NTT):
                for dh in range(2):
                    nc.tensor.matmul(
                        pfs[i][:, ts(dh, 512)],
                        lhsT=ht_tiles[jt][:, ts(i, 128)],
                        rhs=w2t[:, ts(dh, 512)],
                        start=(jt == 0),
                        stop=(jt == FF // 128 - 1),
                    )
        for i in range(NTT):
            # half-width adds + writes so the output DMA starts draining
            # while the second half is still being summed
            for dh in range(2):
                nc.vector.tensor_add(
                    out=out_sb[i][:, ts(dh, 512)],
                    in0=pfs[i][:, ts(dh, 512)],
                    in1=x2_tiles[i][:, ts(dh, 512)],
                )
                nc.sync.dma_start(
                    out=out[ts(i, 128), ds(512 * dh, 512)],
                    in_=out_sb[i][:, ts(dh, 512)],
                )

        f_ps.release()
        outsp.release()
        w2sp.release()
        htp.release()
        w1pb.release()
        ytp.release()
        x2p.release()
        xln.release()
        stat.release()
        w1pa.release()
        embs.release()
        dram.release()
        const.release()
    nc.finalize()
    return nc


_NC = None


def _get_nc():
    global _NC
    if _NC is None:
        _NC = build()
    return _NC


def make_in_maps(embds, Wq, Wk, Wv, ln1_g, ln1_b, ln2_g, ln2_b, W1, b1, W2, b2):
    embds = np.ascontiguousarray(np.asarray(embds, dtype=np.float32)).reshape(NT, D)
    Wq = np.asarray(Wq, dtype=np.float32)
    Wk = np.asarray(Wk, dtype=np.float32)
    Wv = np.asarray(Wv, dtype=np.float32)
    W1 = np.ascontiguousarray(np.asarray(W1, dtype=np.float32))
    W2 = np.ascontiguousarray(np.asarray(W2, dtype=np.float32))
    b1 = np.asarray(b1, dtype=np.float32)
    b2 = np.asarray(b2, dtype=np.float32)
    g1 = np.asarray(ln1_g, dtype=np.float32)
    bb1 = np.asarray(ln1_b, dtype=np.float32)
    g2 = np.asarray(ln2_g, dtype=np.float32)
    bb2 = np.asarray(ln2_b, dtype=np.float32)

    # Fold LN1 gain/bias into the QKV projections:
    #   q = (xn*g1 + b1) @ Wq = xn @ (g1[:,None]*Wq) + b1@Wq
    Wqf = Wq * g1[None, :, None]
    Wkf = Wk * g1[None, :, None]
    Wvf = Wv * g1[None, :, None]
    bq = np.einsum("d,hde->he", bb1, Wq)  # [H, HS]
    bk = np.einsum("d,hde->he", bb1, Wk)
    bv = np.einsum("d,hde->he", bb1, Wv)

    # Fold LN2 gain/bias into the FFN up-projection:
    #   h_pre = (yn*g2 + b2ln) @ W1 + b1 = yn @ (g2[:,None]*W1) + (b2ln@W1 + b1)
    W1f = (W1 * g2[:, None]).astype(ml_dtypes.bfloat16)
    b1f = b1 + bb2 @ W1
    W2b = W2.astype(ml_dtypes.bfloat16)
    b1r = np.ascontiguousarray(b1f.reshape(FF // 128, 128).T.astype(np.float32))

    def _w_slice(W, c):
        # heads (2c, 2c+1): [2, D, HS] -> [D, 2*HS] -> [128, 1024]
        s = W[2 * c : 2 * c + 2].transpose(1, 0, 2).reshape(D, 2 * HS)
        r = np.ascontiguousarray(s.reshape(8, 128, 2 * HS).transpose(1, 0, 2))
        return r.reshape(128, 1024).astype(ml_dtypes.bfloat16)

    in_maps = []
    for c in range(NCORES):
        bqkv = np.stack(
            [
                np.concatenate([bq[2 * c], bq[2 * c + 1]]),
                np.concatenate([bk[2 * c], bk[2 * c + 1]]),
                np.concatenate([bv[2 * c], bv[2 * c + 1]]),
            ],
            axis=1,
        ).astype(np.float32)  # [128, 3]
        in_maps.append(
            {
                "emb": np.ascontiguousarray(embds[c * TC : (c + 1) * TC]),
                "wq": _w_slice(Wqf, c),
                "wk": _w_slice(Wkf, c),
                "wv": _w_slice(Wvf, c),
                "bqkv": np.ascontiguousarray(bqkv),
                "w1": W1f,
                "w2": W2b,
                "b1r": b1r,
                "b2f": np.ascontiguousarray(b2),
            }
        )
    return in_maps


def run(in_maps, trace=False, **kwargs):
    from concourse.bass_utils import run_bass_kernel_spmd

    nc = _get_nc()
    return run_bass_kernel_spmd(
        nc, in_maps, core_ids=list(range(NCORES)), trace=trace, **kwargs
    )


def kernel(**inputs):
    in_maps = make_in_maps(**inputs)
    res = run(in_maps, trace=False)
    outs = [res.results[c]["out"] for c in range(NCORES)]
    return np.concatenate(outs, axis=0).reshape(B, T, D)


# revision 15
# speedup vs baseline: 1.0863x; 1.0863x over previous
"""Fused transformer block (LN1 -> 16-head causal attention -> LN2 -> FFN,
two residuals) on 8 Trainium2 NeuronCores.

Sharding strategy
-----------------
- Attention is head-parallel: core c owns heads (2c, 2c+1) and computes their
  Q^T/K^T/V^T and attention output O^T for ALL 4096 tokens (B*T flattened).
- LN1 + transpose of the normalized activations is sequence-parallel (512
  tokens per core), stitched with an AllGather of x^T (feature-major).
- An AllToAll converts the head-parallel attention output into token-parallel
  layout; residual + LN2 + the whole FFN then run sequence-parallel (512
  tokens per core) with zero further communication. Each core returns its
  512-token slice of the output; the host concatenates.

Performance notes:
- LN gains/biases are folded into the weights host-side: Wq/Wk/Wv absorb
  ln1_g (per-head bias vectors absorb ln1_b), W1 absorbs ln2_g and b1
  absorbs ln2_b@W1. LN on device is just (x - mean) * rstd.
- A tiny dummy AllGather fires at kernel start to absorb the ~10us ncfw
  first-collective warmup, so the real AllGather begins ~0.5us after its
  trigger.
- The x^T AllGather is split into two feature-halves; QKV accumulation is
  split k0-3 / k4-7 so the second half of the collective overlaps the first
  half's matmuls (2 chunks in flight, 6 PSUM banks).
- wq/wk/wv travel as contiguous [128, 1024] rows (one 2KB descriptor per
  partition) so the DMA queues clear early and the emb tiles land fast.
- emb is loaded once; the same SBUF tiles feed LN1 and the phase-H residual.
- Causal masking is an in-place affine_select on the (otherwise idle) GpSimd
  engine, replacing per-tile Vector multiplies and the mask tiles.
- W1 is preloaded at the start of attention (DMA queues are idle there) so
  the 8 MB of reads never contend with the AllToAll.
- Attention is software-pipelined across chunks: scores+exp of chunk c+1 are
  emitted before the PV accumulation of chunk c, so the PE always has
  runnable work while the Scalar engine drains the softmax exps. Two score
  tiles share a 2-bank PSUM tile so one Exp covers 1024 columns.
- The attention-output AllToAll travels in bf16 (halves the exchange).
"""

import sys

if "/opt/trn_rl_repo" not in sys.path:
    sys.path.insert(0, "/opt/trn_rl_repo")

import ml_dtypes
import numpy as np

import concourse.bass as bass
import concourse.mybir as mybir
import concourse.tile as tile
from concourse import bacc
from concourse.bass import ds, ts
from concourse.masks import make_identity

# ── Problem constants (hardcoded; see spec) ──────────────────────────────────
B, T, D = 2, 2048, 1024
H, HS = 16, 64
FF = 4 * D  # 4096
EPS = 1e-5
NCORES = 8
NT = B * T  # 4096 tokens
TC = NT // NCORES  # 512 tokens per core
NTT = TC // 128  # 4 token tiles per core
HPC = H // NCORES  # 2 heads per core
SCALE = 1.0 / float(np.sqrt(D))  # reference scales scores by D**-0.5

F32 = mybir.dt.float32
BF16 = mybir.dt.bfloat16
AF = mybir.ActivationFunctionType
OP = mybir.AluOpType


def build():
    nc = bacc.Bacc(num_devices=NCORES)

    emb = nc.dram_tensor("emb", [TC, D], F32, kind="ExternalInput")
    # per-core head slice of Wq/Wk/Wv (ln1_g folded in), contiguous
    # [128(d_in), 1024(8 d_tiles x 2*HS)] rows -> one 2KB descriptor per row
    wq = nc.dram_tensor("wq", [128, 1024], BF16, kind="ExternalInput")
    wk = nc.dram_tensor("wk", [128, 1024], BF16, kind="ExternalInput")
    wv = nc.dram_tensor("wv", [128, 1024], BF16, kind="ExternalInput")
    # per-core QKV bias columns (ln1_b folded through the projections)
    bqkv = nc.dram_tensor("bqkv", [128, 3], F32, kind="ExternalInput")
    w1 = nc.dram_tensor("w1", [D, FF], BF16, kind="ExternalInput")
    w2 = nc.dram_tensor("w2", [FF, D], BF16, kind="ExternalInput")
    b1r = nc.dram_tensor("b1r", [128, FF // 128], F32, kind="ExternalInput")
    b2f = nc.dram_tensor("b2f", [D], F32, kind="ExternalInput")
    out = nc.dram_tensor("out", [TC, D], F32, kind="ExternalOutput")

    rg = [list(range(NCORES))]

    with tile.TileContext(nc) as tc:
        # Pools are allocated just-in-time and released LIFO per (space, side).
        const = tc.alloc_tile_pool(name="const", bufs=1)
        dram = tc.alloc_tile_pool(name="dram", bufs=1, space="DRAM")
        stat = tc.alloc_tile_pool(name="stat", bufs=4)
        xln = tc.alloc_tile_pool(name="xln", bufs=2)
        embs = tc.alloc_tile_pool(name="embs", bufs=1, side="right")

        # DRAM bounce buffers
        cc_x_in = dram.tile([D, TC], BF16, name="cc_x_in")
        xg = dram.tile([NCORES * D, TC], BF16, name="xg", addr_space="Shared")
        cc_a_in = dram.tile([NCORES, 130, TC], BF16, name="cc_a_in")
        cc_a_out = dram.tile([NCORES, 130, TC], BF16, name="cc_a_out")

        identity = const.tile([128, 128], F32, name="identity")
        make_identity(nc, identity)
        identity_b = const.tile([128, 128], BF16, name="identity_b")
        nc.vector.tensor_copy(out=identity_b, in_=identity)
        eps_t = const.tile([128, 1], F32, name="eps_t")
        nc.vector.memset(eps_t, EPS)
        b1s = const.tile([128, FF // 128], F32, name="b1s")
        nc.sync.dma_start(out=b1s, in_=b1r[:, :])
        b2b = const.tile([128, D], F32, name="b2b")
        b2a = b2f[:]
        nc.sync.dma_start(
            out=b2b,
            in_=bass.AP(tensor=b2a.tensor, offset=b2a.offset, ap=[[0, 128], [1, D]]),
        )
        bqs = const.tile([128, 3], F32, name="bqs")
        nc.sync.dma_start(out=bqs, in_=bqkv[:, :])

        # attention-lifetime pools (released after the AllToAll)
        qkres = tc.alloc_tile_pool(name="qkres", bufs=1)
        vsbp = tc.alloc_tile_pool(name="vsbp", bufs=1)
        otp = tc.alloc_tile_pool(name="otp", bufs=2)
        ptp = tc.alloc_tile_pool(name="ptp", bufs=6)
        attnc = tc.alloc_tile_pool(name="attnc", bufs=1)

        qT = qkres.tile([128, NT], BF16, name="qT")
        kT = qkres.tile([128, NT], BF16, name="kT")
        v_sb = vsbp.tile([128, NT // 128, HPC, 65], BF16, name="v_sb")
        ones_f = attnc.tile([128, 64], F32, name="ones_f")
        nc.vector.memset(ones_f, 1.0)

        # QKV-lifetime pools
        vtp = tc.alloc_tile_pool(name="vtp", bufs=1)
        wqkvc = tc.alloc_tile_pool(name="wqkvc", bufs=1)
        xrhs = tc.alloc_tile_pool(name="xrhs", bufs=2)
        vT = vtp.tile([128, NT], BF16, name="vT")
        wqs = wqkvc.tile([128, 1024], BF16, name="wqs")
        nc.sync.dma_start(out=wqs, in_=wq[:, :])
        wks = wqkvc.tile([128, 1024], BF16, name="wks")
        nc.sync.dma_start(out=wks, in_=wk[:, :])
        wvs = wqkvc.tile([128, 1024], BF16, name="wvs")
        nc.sync.dma_start(out=wvs, in_=wv[:, :])
        # W1 row-blocks 0-4 preload during attention (right-side stack, so the
        # late release keeps LIFO order); blocks 5-7 load in phase H.
        w1pa = tc.alloc_tile_pool(name="w1pa", bufs=1, side="right")

        def layer_norm(src_tile, dst_tile, use_scalar=False):
            """dst = (src - mean) * rstd  (per 128-token tile, stats over D).

            LN gain/bias are folded into the downstream weights host-side.
            With use_scalar the wide normalize pass runs on the Scalar engine
            as Identity(src * rstd + (-mean * rstd)), halving the Vector load.
            """
            st = stat.tile([128, 2, 6], F32, name="st", tag="st")
            nc.vector.bn_stats(out=st[:, 0, :], in_=src_tile[:, 0:512])
            nc.vector.bn_stats(out=st[:, 1, :], in_=src_tile[:, 512:1024])
            mv = stat.tile([128, 2], F32, name="mv", tag="mv")
            nc.vector.bn_aggr(out=mv, in_=st)
            std = stat.tile([128, 1], F32, name="std", tag="std")
            nc.scalar.activation(
                out=std, in_=mv[:, 1:2], func=AF.Sqrt, bias=eps_t, scale=1.0
            )
            rstd = stat.tile([128, 1], F32, name="rstd", tag="rstd")
            nc.vector.reciprocal(out=rstd, in_=std)
            if use_scalar:
                nm = stat.tile([128, 1], F32, name="nm", tag="nm")
                nc.vector.tensor_scalar(
                    out=nm,
                    in0=mv[:, 0:1],
                    scalar1=rstd,
                    scalar2=-1.0,
                    op0=OP.mult,
                    op1=OP.mult,
                )
                nc.scalar.activation(
                    out=dst_tile, in_=src_tile, func=AF.Identity, bias=nm, scale=rstd
                )
            else:
                nc.vector.tensor_scalar(
                    out=dst_tile,
                    in0=src_tile,
                    scalar1=mv[:, 0:1],
                    scalar2=rstd,
                    op0=OP.subtract,
                    op1=OP.mult,
                )

        # ── Phase A+B: LN1 on own 512-token chunk, transpose, send to AGs ──
        # emb tiles are persistent: they feed LN1 here and the attn residual
        # in phase H (loaded once).
        xtp = tc.alloc_tile_pool(name="xtp", bufs=1)
        ab_tp = tc.alloc_tile_pool(name="ab_tp", bufs=4, space="PSUM")
        xt_tiles = [xtp.tile([128, TC], BF16, name=f"xt{k}") for k in range(8)]
        et_tiles = []
        for i in range(NTT):
            et = embs.tile([128, D], F32, name="et", tag=f"et{i}", bufs=1)
            nc.sync.dma_start(out=et, in_=emb[ts(i, 128), :])
            et_tiles.append(et)
        xn_tiles = []
        for i in range(NTT):
            xn = xln.tile([128, D], F32, name="xn", tag="xn", bufs=4)
            layer_norm(et_tiles[i], xn, use_scalar=(i % 2 == 1))
            xn_tiles.append(xn)
        # k-major transposes with a per-block DMA: each feature block of the
        # AllGather input ships as soon as it is complete, so the collective's
        # data dependency clears right after the last block instead of after
        # a bulk 8-DMA tail
        for k in range(8):
            for i in range(NTT):
                ps = ab_tp.tile([128, 128], F32, name="abtp", tag="abtp")
                nc.tensor.transpose(ps, xn_tiles[i][:, ts(k, 128)], identity)
                if k % 2 == 0:
                    nc.vector.tensor_copy(out=xt_tiles[k][:, ts(i, 128)], in_=ps)
                else:
                    nc.scalar.copy(out=xt_tiles[k][:, ts(i, 128)], in_=ps)
            nc.sync.dma_start(out=cc_x_in[ts(k, 128), :], in_=xt_tiles[k])

        # ── Phase C: AllGather x^T chunks ───────────────────────────────
        nc.gpsimd.collective_compute(
            "AllGather", OP.bypass, replica_groups=rg, ins=[cc_x_in.opt()], outs=[xg.opt()]
        )
        xtp.release()
        ab_tp.release()

        # ones rows of V (the denominator trick) are constant: fill them once
        # before phase D so the per-chunk work below only writes the V data
        nc.vector.tensor_copy(
            out=v_sb[:, :, :, 64:65],
            in_=ones_f[:, 0:64].rearrange("p (a b c) -> p a b c", a=NT // 128, b=HPC),
        )

        # ── Phase D+E: Q^T/K^T/V^T per chunk, V transposes inlined ──────
        # The V^T->V transposes for chunk cb are emitted right after its
        # drains, so the PE never has a transpose-only stretch (transposes
        # don't count as activity for the HAM clock monitor, and a stretch
        # of them lets the PE clock drop to 1.2 GHz).
        qkv_ps = tc.alloc_tile_pool(name="qkv_ps", bufs=2, space="PSUM")
        for cb in range(NCORES):
            xr = xrhs.tile([128, 8, TC], BF16, name="xr", tag="xr")
            xga = xg[:]
            if cb == 0:
                # first chunk: per-block DMAs so the k=0 matmul starts as
                # soon as its 128 KB lands instead of after the full MB
                for k in range(8):
                    src = bass.AP(
                        tensor=xga.tensor,
                        offset=xga.offset + 128 * k * TC,
                        ap=[[TC, 128], [1, TC]],
                    )
                    nc.sync.dma_start(out=xr[:, k, :], in_=src)
            else:
                # one 3-D DMA brings in all 8 feature blocks of this chunk
                src = bass.AP(
                    tensor=xga.tensor,
                    offset=xga.offset + D * cb * TC,
                    ap=[[TC, 128], [128 * TC, 8], [1, TC]],
                )
                nc.sync.dma_start(out=xr, in_=src)
            pq = qkv_ps.tile([128, TC], F32, name="pq", tag="pq")
            pk = qkv_ps.tile([128, TC], F32, name="pk", tag="pk")
            pv = qkv_ps.tile([128, TC], F32, name="pv", tag="pv")
            for k in range(8):
                kw = dict(start=(k == 0), stop=(k == 7))
                nc.tensor.matmul(pq, lhsT=wqs[:, ts(k, 128)], rhs=xr[:, k, :], **kw)
                nc.tensor.matmul(pk, lhsT=wks[:, ts(k, 128)], rhs=xr[:, k, :], **kw)
                nc.tensor.matmul(pv, lhsT=wvs[:, ts(k, 128)], rhs=xr[:, k, :], **kw)
            nc.vector.tensor_scalar_add(
                out=qT[:, ts(cb, TC)], in0=pq, scalar1=bqs[:, 0:1]
            )
            nc.vector.tensor_scalar_add(
                out=kT[:, ts(cb, TC)], in0=pk, scalar1=bqs[:, 1:2]
            )
            nc.scalar.activation(
                out=vT[:, ts(cb, TC)], in_=pv, func=AF.Identity,
                bias=bqs[:, 2:3], scale=1.0,
            )
            if cb >= 1:
                # V transposes for the PREVIOUS chunk (its vT is drained);
                # interleaved between chunk matmul groups
                for st_ in range(4 * (cb - 1), 4 * cb):
                    ps = qkv_ps.tile([128, 128], BF16, name="tpb", tag="tpb")
                    nc.tensor.transpose(ps, vT[:, ts(st_, 128)], identity_b)
                    nc.vector.tensor_copy(
                        out=v_sb[:, st_, :, 0:64],
                        in_=ps.rearrange("p (h e) -> p h e", h=HPC),
                    )
        for st_ in range(4 * (NCORES - 1), 4 * NCORES):
            ps = qkv_ps.tile([128, 128], BF16, name="tpb", tag="tpb")
            nc.tensor.transpose(ps, vT[:, ts(st_, 128)], identity_b)
            nc.vector.tensor_copy(
                out=v_sb[:, st_, :, 0:64],
                in_=ps.rearrange("p (h e) -> p h e", h=HPC),
            )
        xrhs.release()
        wqkvc.release()
        vtp.release()
        qkv_ps.release()

        # ── Phase F: causal attention, software-pipelined across chunks ──
        # Per chunk: S^T matmuls stream through PSUM banks, exp evacuates to
        # bf16 SBUF; causal masking is an in-place affine_select on GpSimd.
        # The scores+exp of chunk c+1 are emitted BEFORE the PV accumulation
        # of chunk c so the PE has dependency-free work while Scalar drains.
        s_ps = tc.alloc_tile_pool(name="s_ps", bufs=3, space="PSUM")
        o_ps = tc.alloc_tile_pool(name="o_ps", bufs=1, space="PSUM")

        # W1 partial preload: DMA queues are idle during attention, so 5 MB of
        # the reads finish long before the AllToAll needs the wires.
        w1sb = [w1pa.tile([128, FF], BF16, name=f"w1sb{k}") for k in range(5)]
        for k in range(5):
            nc.sync.dma_start(out=w1sb[k], in_=w1[ts(k, 128), :])

        def build_score_ops(gc, pts):
            """Closures, each emitting one scores pair: 2 mms + exp (+ mask).

            Two s-tiles share one 2-bank PSUM tile so a single Exp covers
            1024 columns (halves the Scalar per-instruction overhead).
            Causal column-skipping: a diag-d tile's first 128*d query
            columns never survive the mask, so every stage (scores, exp,
            mask, PV) is restricted to the columns the next stage reads.
            """
            b = gc // 4
            lc = gc % 4
            nst = 4 * lc + 4  # s-tiles (128 wide) within this batch
            t0g = gc * TC
            ops = []
            for sp in range(nst // 2):
                for h in range(HPC):
                    def op(sp=sp, h=h):
                        d0 = 2 * sp - 4 * lc
                        cl = 128 * d0 if d0 > 0 else 0
                        ps_ = s_ps.tile([128, 2, TC], F32, name="ps_", tag="ps_")
                        pt_ = ptp.tile(
                            [128, 2, TC], BF16, name="pt_", tag="pt_", bufs=29
                        )
                        for hf in range(2):
                            stl = 2 * sp + hf
                            sg = b * 16 + stl
                            diag = stl - 4 * lc
                            c0 = 128 * diag if diag > 0 else 0
                            # heads use PE row-groups 0-63 / 64-127
                            nc.tensor.matmul(
                                ps_[:, hf, ds(c0, TC - c0)],
                                lhsT=kT[ts(h, 64), ts(sg, 128)],
                                rhs=qT[ts(h, 64), ds(t0g + c0, TC - c0)],
                                start=True,
                                stop=True,
                            )
                        nc.scalar.activation(
                            out=pt_[:, :, ds(cl, TC - cl)],
                            in_=ps_[:, :, ds(cl, TC - cl)],
                            func=AF.Exp,
                            scale=SCALE,
                        )
                        if d0 + 1 >= 0:
                            # causal triangle of the diagonal pair, zeroed in
                            # place on GpSimd: keep where col >= 128*diag+row
                            nc.gpsimd.affine_select(
                                out=pt_[:, :, ds(cl, TC - cl)],
                                in_=pt_[:, :, ds(cl, TC - cl)],
                                pattern=[[-128, 2], [1, TC - cl]],
                                compare_op=OP.is_ge,
                                fill=0.0,
                                base=cl - 128 * d0,
                                channel_multiplier=-1,
                            )
                        for hf in range(2):
                            stl = 2 * sp + hf
                            diag = stl - 4 * lc
                            c0 = 128 * diag if diag > 0 else 0
                            pts[(stl, h)] = (pt_, hf, c0)
                    ops.append(op)
            return ops

        def build_pv_ops(gc, pts):
            """Closures: PV matmuls head-alternating, then drains + DMA."""
            b = gc // 4
            lc = gc % 4
            nst = 4 * lc + 4
            po = {}
            oT_h = [None]
            ops = []

            def mm(h, stl):
                if stl == 0:
                    po[h] = o_ps.tile([65, TC], F32, name=f"po{h}", tag=f"po{h}", bufs=1)
                sg = b * 16 + stl
                pt_, hf, c0 = pts[(stl, h)]
                nc.tensor.matmul(
                    po[h][:, ds(c0, TC - c0)],
                    lhsT=v_sb[:, sg, h, :],
                    rhs=pt_[:, hf, ds(c0, TC - c0)],
                    start=(stl == 0),
                    stop=(stl == nst - 1),
                    skip_group_check=(c0 > 0),
                )

            def finish(h):
                if oT_h[0] is None:
                    oT_h[0] = otp.tile([128, TC], BF16, name="oT", tag="oT")
                nc.vector.tensor_copy(out=oT_h[0][ts(h, 64), :], in_=po[h][0:64, :])
                oTd = otp.tile([1, TC], BF16, name="oTd", tag="oTd")
                nc.vector.tensor_copy(out=oTd, in_=po[h][64:65, :])
                nc.sync.dma_start(out=cc_a_in[gc, 128 + h : 129 + h, :], in_=oTd)
                if h == HPC - 1:
                    nc.sync.dma_start(out=cc_a_in[gc, 0:128, :], in_=oT_h[0])

            for stl in range(nst):
                for h in range(HPC):
                    ops.append(lambda h=h, stl=stl: mm(h, stl))
            for h in range(HPC):
                ops.append(lambda h=h: finish(h))
            return ops

        # Fine-grained interleave: scores pairs of chunk c+1 are woven between
        # the PV matmuls of chunk c, so the PE's instruction stream always has
        # 128-contraction matmuls in flight (keeps the HAM clock monitor at
        # full rate — a long stretch of 64-row scores reads as "idle" to it)
        # and the Scalar engine's exp latency is hidden.
        pend = []
        for gc in range(NCORES):
            pts = {}
            s_ops = build_score_ops(gc, pts)
            ns, npv = len(s_ops), len(pend)
            pi = 0
            for si in range(ns):
                s_ops[si]()
                target = (npv * (si + 1)) // ns
                while pi < target:
                    pend[pi]()
                    pi += 1
            while pi < npv:
                pend[pi]()
                pi += 1
            pend = build_pv_ops(gc, pts)
        for op in pend:
            op()

        # ── Phase G: AllToAll -> unnormalized attn^T + denoms, own tokens ──
        nc.gpsimd.collective_compute(
            "AllToAll", OP.bypass, replica_groups=rg, ins=[cc_a_in.opt()], outs=[cc_a_out.opt()]
        )
        o_ps.release()
        s_ps.release()
        attnc.release()
        ptp.release()
        otp.release()
        vsbp.release()
        qkres.release()

        # ── Phase H: normalize + attn residual + LN2, y -> y^T ──────────
        asbp = tc.alloc_tile_pool(name="asbp", bufs=4, side="right")
        h_tp = tc.alloc_tile_pool(name="h_tp", bufs=5, space="PSUM")
        x2p = tc.alloc_tile_pool(name="x2p", bufs=1)
        ytp = tc.alloc_tile_pool(name="ytp", bufs=1)
        w1pb = tc.alloc_tile_pool(name="w1pb", bufs=1)
        x2_tiles = [x2p.tile([128, D], F32, name=f"x2_{i}") for i in range(NTT)]
        yt_tiles = [ytp.tile([128, TC], BF16, name=f"yt{k}") for k in range(8)]
        asb_tiles = []
        for c in range(NCORES):
            asb = asbp.tile([128, TC], BF16, name="asb", tag="asb", bufs=8)
            nc.sync.dma_start(out=asb, in_=cc_a_out[c, 0:128, :])
            dnm = asbp.tile([2, TC], BF16, name="dnm", tag="dnm", bufs=8)
            nc.sync.dma_start(out=dnm, in_=cc_a_out[c, 128:130, :])
            asb_tiles.append((asb, dnm))
        # W1 row-blocks 5-7: queued behind the asb DMAs, so they drain right
        # after the AllToAll completes instead of contending with it.
        for k in range(5, 8):
            w1sb.append(w1pb.tile([128, FF], BF16, name=f"w1sb{k}"))
            nc.sync.dma_start(out=w1sb[k], in_=w1[ts(k, 128), :])
        # i-major: each x2 tile completes as early as possible so its LN2
        # (emitted right after) overlaps the remaining residual work
        yn_tiles = []
        for i in range(NTT):
            for c in range(NCORES):
                asb, dnm = asb_tiles[c]
                pn = h_tp.tile([128, 128], BF16, name="htp", tag="htp")
                nc.tensor.transpose(pn, asb[:, ts(i, 128)], identity_b)
                pd = h_tp.tile([128, 2], BF16, name="hpd", tag="htp")
                nc.tensor.transpose(pd, dnm[:, ts(i, 128)], identity_b[0:2, 0:2])
                rcp = asbp.tile([128, 2], F32, name="rcp", tag="rcp")
                nc.vector.reciprocal(out=rcp, in_=pd)
                for h in range(HPC):
                    nc.vector.scalar_tensor_tensor(
                        out=x2_tiles[i][:, ds(128 * c + 64 * h, 64)],
                        in0=pn[:, ts(h, 64)],
                        scalar=rcp[:, h : h + 1],
                        in1=et_tiles[i][:, ds(128 * c + 64 * h, 64)],
                        op0=OP.mult,
                        op1=OP.add,
                    )
            # shares the xn tag: the phase-A xn tiles are dead by now, and
            # reusing their buffers keeps the SBUF budget under the limit
            yn = xln.tile([128, D], F32, name="yn", tag="xn", bufs=4)
            layer_norm(x2_tiles[i], yn, use_scalar=(i % 2 == 1))
            yn_tiles.append(yn)
        # k-major transposes: yt[0] (which gates the FFN's first matmul)
        # completes first instead of last
        for k in range(8):
            for i in range(NTT):
                ps = h_tp.tile([128, 128], F32, name="htp2", tag="htp2", bufs=3)
                nc.tensor.transpose(ps, yn_tiles[i][:, ts(k, 128)], identity)
                if k % 2 == 0:
                    nc.vector.tensor_copy(out=yt_tiles[k][:, ts(i, 128)], in_=ps)
                else:
                    nc.scalar.copy(out=yt_tiles[k][:, ts(i, 128)], in_=ps)
        asbp.release()
        h_tp.release()

        # ── Phase J: FFN up-projection, h^T = relu(W1^T y^T + b1) ───────
        htp = tc.alloc_tile_pool(name="htp", bufs=1)
        w2sp = tc.alloc_tile_pool(name="w2sp", bufs=6)
        outsp = tc.alloc_tile_pool(name="outs", bufs=1)
        h_ps = tc.alloc_tile_pool(name="h_ps", bufs=4, space="PSUM")
        ht_tiles = [htp.tile([128, TC], BF16, name=f"ht{j}") for j in range(FF // 128)]
        out_sb = [outsp.tile([128, D], F32, name=f"osb{i}") for i in range(NTT)]
        # fold the down-projection bias into the residual while Vector is idle
        # (LN2 has already consumed x2, so this is safe)
        for i in range(NTT):
            nc.vector.tensor_add(out=x2_tiles[i], in0=x2_tiles[i], in1=b2b)
        for jg in range(16):
            phs = [h_ps.tile([128, TC], F32, name=f"ph{jj}", tag="ph") for jj in range(2)]
            for k in range(8):
                for jj in range(2):
                    nc.tensor.matmul(
                        phs[jj],
                        lhsT=w1sb[k][:, ds(256 * jg + 128 * jj, 128)],
                        rhs=yt_tiles[k],
                        start=(k == 0),
                        stop=(k == 7),
                    )
            for jj in range(2):
                jt = 2 * jg + jj
                nc.scalar.activation(
                    out=ht_tiles[jt],
                    in_=phs[jj],
                    func=AF.Relu,
                    bias=b1s[:, jt : jt + 1],
                    scale=1.0,
                )
        h_ps.release()

        # ── Phase K: FFN down-projection, natural [token, D] accumulation ──
        # lhsT is an h^T chunk reused for both 512-wide halves of W2's rows;
        # each token tile owns a 2-bank PSUM accumulator, so the output needs
        # no final transposes — just one residual add per tile.
        f_ps = tc.alloc_tile_pool(name="f_ps", bufs=4, space="PSUM")
        pfs = [f_ps.tile([128, D], F32, name=f"pf{i}", tag="pf") for i in range(NTT)]
        for jt in range(FF // 128):
            w2t = w2sp.tile([128, D], BF16, name="w2t", tag="w2t")
            nc.sync.dma_start(out=w2t, in_=w2[ts(jt, 128), :])
            for i in range(NTT):
                for dh in range(2):
                    nc.tensor.matmul(
                        pfs[i][:, ts(dh, 512)],
                        lhsT=ht_tiles[jt][:, ts(i, 128)],
                        rhs=w2t[:, ts(dh, 512)],
                        start=(jt == 0),
                        stop=(jt == FF // 128 - 1),
                    )
        for i in range(NTT):
            # half-width adds + writes so the output DMA starts draining
            # while the second half is still being summed
            for dh in range(2):
                nc.vector.tensor_add(
                    out=out_sb[i][:, ts(dh, 512)],
                    in0=pfs[i][:, ts(dh, 512)],
                    in1=x2_tiles[i][:, ts(dh, 512)],
                )
                nc.sync.dma_start(
                    out=out[ts(i, 128), ds(512 * dh, 512)],
                    in_=out_sb[i][:, ts(dh, 512)],
                )

        f_ps.release()
        outsp.release()
        w2sp.release()
        htp.release()
        w1pb.release()
        ytp.release()
        x2p.release()
        xln.release()
        stat.release()
        w1pa.release()
        embs.release()
        dram.release()
        const.release()
    nc.finalize()
    return nc


_NC = None


def _get_nc():
    global _NC
    if _NC is None:
        _NC = build()
    return _NC


def make_in_maps(embds, Wq, Wk, Wv, ln1_g, ln1_b, ln2_g, ln2_b, W1, b1, W2, b2):
    embds = np.ascontiguousarray(np.asarray(embds, dtype=np.float32)).reshape(NT, D)
    Wq = np.asarray(Wq, dtype=np.float32)
    Wk = np.asarray(Wk, dtype=np.float32)
    Wv = np.asarray(Wv, dtype=np.float32)
    W1 = np.ascontiguousarray(np.asarray(W1, dtype=np.float32))
    W2 = np.ascontiguousarray(np.asarray(W2, dtype=np.float32))
    b1 = np.asarray(b1, dtype=np.float32)
    b2 = np.asarray(b2, dtype=np.float32)
    g1 = np.asarray(ln1_g, dtype=np.float32)
    bb1 = np.asarray(ln1_b, dtype=np.float32)
    g2 = np.asarray(ln2_g, dtype=np.float32)
    bb2 = np.asarray(ln2_b, dtype=np.float32)

    # Fold LN1 gain/bias into the QKV projections:
    #   q = (xn*g1 + b1) @ Wq = xn @ (g1[:,None]*Wq) + b1@Wq
    Wqf = Wq * g1[None, :, None]
    Wkf = Wk * g1[None, :, None]
    Wvf = Wv * g1[None, :, None]
    bq = np.einsum("d,hde->he", bb1, Wq)  # [H, HS]
    bk = np.einsum("d,hde->he", bb1, Wk)
    bv = np.einsum("d,hde->he", bb1, Wv)

    # Fold LN2 gain/bias into the FFN up-projection:
    #   h_pre = (yn*g2 + b2ln) @ W1 + b1 = yn @ (g2[:,None]*W1) + (b2ln@W1 + b1)
    W1f = (W1 * g2[:, None]).astype(ml_dtypes.bfloat16)
    b1f = b1 + bb2 @ W1
    W2b = W2.astype(ml_dtypes.bfloat16)
    b1r = np.ascontiguousarray(b1f.reshape(FF // 128, 128).T.astype(np.float32))

    def _w_slice(W, c):
        # heads (2c, 2c+1): [2, D, HS] -> [D, 2*HS] -> [128, 1024]
        s = W[2 * c : 2 * c + 2].transpose(1, 0, 2).reshape(D, 2 * HS)
        r = np.ascontiguousarray(s.reshape(8, 128, 2 * HS).transpose(1, 0, 2))
        return r.reshape(128, 1024).astype(ml_dtypes.bfloat16)

    in_maps = []
    for c in range(NCORES):
        bqkv = np.stack(
            [
                np.concatenate([bq[2 * c], bq[2 * c + 1]]),
                np.concatenate([bk[2 * c], bk[2 * c + 1]]),
                np.concatenate([bv[2 * c], bv[2 * c + 1]]),
            ],
            axis=1,
        ).astype(np.float32)  # [128, 3]
        in_maps.append(
            {
                "emb": np.ascontiguousarray(embds[c * TC : (c + 1) * TC]),
                "wq": _w_slice(Wqf, c),
                "wk": _w_slice(Wkf, c),
                "wv": _w_slice(Wvf, c),
                "bqkv": np.ascontiguousarray(bqkv),
                "w1": W1f,
                "w2": W2b,
                "b1r": b1r,
                "b2f": np.ascontiguousarray(b2),
            }
        )
    return in_maps


def run(in_maps, trace=False, **kwargs):
    from concourse.bass_utils import run_bass_kernel_spmd

    nc = _get_nc()
    return run_bass_kernel_spmd(
        nc, in_maps, core_ids=list(range(NCORES)), trace=trace, **kwargs
    )


def kernel(**inputs):
    in_maps = make_in_maps(**inputs)
    res = run(in_maps, trace=False)
    outs = [res.results[c]["out"] for c in range(NCORES)]
    return np.concatenate(outs, axis=0).reshape(B, T, D)


# revision 20
# speedup vs baseline: 1.0962x; 1.0091x over previous
"""Fused transformer block (LN1 -> 16-head causal attention -> LN2 -> FFN,
two residuals) on 8 Trainium2 NeuronCores.

Sharding strategy
-----------------
- QKV is sequence-parallel: each core LN1s its OWN 512 tokens and computes
  q/k/v for ALL 16 heads from the local x^T — no collective needed first,
  so the ~65us collectives-subsystem warmup window is filled with matmuls.
- An AllToAll (384 KB/slot) then redistributes q/k/v head-wise: core c
  receives q^T/k^T/v for its 2 heads (2c, 2c+1) over ALL 4096 tokens.
- Attention is head-parallel; a second AllToAll converts the attention
  output back to token-parallel layout; residual + LN2 + the whole FFN run
  sequence-parallel with zero further communication. Each core returns its
  512-token slice of the output; the host concatenates.

Performance notes:
- LN gains/biases are folded into the weights host-side: Wq/Wk/Wv absorb
  ln1_g (bias vectors absorb ln1_b), W1 absorbs ln2_g and b1 absorbs
  ln2_b@W1. LN on device is just (x - mean) * rstd.
- QKV runs k-outer (one projection at a time, 8 PSUM banks) so its matmuls
  start as soon as the first transposed feature block exists.
- emb is loaded once; the same SBUF tiles feed LN1 and the phase-H residual.
- Causal masking is an in-place affine_select on the (otherwise idle) GpSimd
  engine, replacing per-tile Vector multiplies and the mask tiles.
- W1 is preloaded during attention (DMA queues are idle there) so the 8 MB
  of reads never contend with the AllToAlls.
- Attention is software-pipelined: scores+exp of chunk c+1 are woven between
  the PV matmuls of chunk c at single-matmul granularity, so the PE always
  has 128-contraction work in flight (keeps the HAM clock monitor engaged)
  while the Scalar engine drains the softmax exps. Two score tiles share a
  2-bank PSUM tile so one Exp covers 1024 columns. The V^T->V transposes are
  woven into the first attention chunks the same way.
- Both AllToAlls travel in bf16.
"""

import sys

if "/opt/trn_rl_repo" not in sys.path:
    sys.path.insert(0, "/opt/trn_rl_repo")

import ml_dtypes
import numpy as np

import concourse.bass as bass
import concourse.mybir as mybir
import concourse.tile as tile
from concourse import bacc
from concourse.bass import ds, ts
from concourse.masks import make_identity

# ── Problem constants (hardcoded; see spec) ──────────────────────────────────
B, T, D = 2, 2048, 1024
H, HS = 16, 64
FF = 4 * D  # 4096
EPS = 1e-5
NCORES = 8
NT = B * T  # 4096 tokens
TC = NT // NCORES  # 512 tokens per core
NTT = TC // 128  # 4 token tiles per core
HPC = H // NCORES  # 2 heads per core
SCALE = 1.0 / float(np.sqrt(D))  # reference scales scores by D**-0.5

F32 = mybir.dt.float32
BF16 = mybir.dt.bfloat16
AF = mybir.ActivationFunctionType
OP = mybir.AluOpType


def build():
    nc = bacc.Bacc(num_devices=NCORES)

    emb = nc.dram_tensor("emb", [TC, D], F32, kind="ExternalInput")
    # full folded projection weights, d_in-major rows (replicated per core)
    wq = nc.dram_tensor("wq", [D, H * HS], BF16, kind="ExternalInput")
    wk = nc.dram_tensor("wk", [D, H * HS], BF16, kind="ExternalInput")
    wv = nc.dram_tensor("wv", [D, H * HS], BF16, kind="ExternalInput")
    # QKV bias columns (ln1_b folded through the projections): [128, 3*8]
    # column p*8+j is the bias for projection p, feature row-block j
    bqkv = nc.dram_tensor("bqkv", [128, 24], F32, kind="ExternalInput")
    w1 = nc.dram_tensor("w1", [D, FF], BF16, kind="ExternalInput")
    w2 = nc.dram_tensor("w2", [FF, D], BF16, kind="ExternalInput")
    b1r = nc.dram_tensor("b1r", [128, FF // 128], F32, kind="ExternalInput")
    b2f = nc.dram_tensor("b2f", [D], F32, kind="ExternalInput")
    out = nc.dram_tensor("out", [TC, D], F32, kind="ExternalOutput")

    rg = [list(range(NCORES))]

    with tile.TileContext(nc) as tc:
        # Pools are allocated just-in-time and released LIFO per (space, side).
        const = tc.alloc_tile_pool(name="const", bufs=1)
        dram = tc.alloc_tile_pool(name="dram", bufs=1, space="DRAM")
        stat = tc.alloc_tile_pool(name="stat", bufs=4)
        xln = tc.alloc_tile_pool(name="xln", bufs=2)
        embs = tc.alloc_tile_pool(name="embs", bufs=1, side="right")

        # DRAM bounce buffers
        cc_q_in = dram.tile([NCORES, 3, 128, TC], BF16, name="cc_q_in")
        cc_q_out = dram.tile([NCORES, 3, 128, TC], BF16, name="cc_q_out")
        cc_a_in = dram.tile([NCORES, 130, TC], BF16, name="cc_a_in")
        cc_a_out = dram.tile([NCORES, 130, TC], BF16, name="cc_a_out")

        identity = const.tile([128, 128], F32, name="identity")
        make_identity(nc, identity)
        identity_b = const.tile([128, 128], BF16, name="identity_b")
        nc.vector.tensor_copy(out=identity_b, in_=identity)
        eps_t = const.tile([128, 1], F32, name="eps_t")
        nc.vector.memset(eps_t, EPS)
        b1s = const.tile([128, FF // 128], F32, name="b1s")
        nc.sync.dma_start(out=b1s, in_=b1r[:, :])
        b2b = const.tile([128, D], F32, name="b2b")
        b2a = b2f[:]
        nc.sync.dma_start(
            out=b2b,
            in_=bass.AP(tensor=b2a.tensor, offset=b2a.offset, ap=[[0, 128], [1, D]]),
        )
        bqs = const.tile([128, 24], F32, name="bqs")
        nc.sync.dma_start(out=bqs, in_=bqkv[:, :])

        def layer_norm(src_tile, dst_tile, use_scalar=False):
            """dst = (src - mean) * rstd  (per 128-token tile, stats over D).

            LN gain/bias are folded into the downstream weights host-side.
            With use_scalar the wide normalize pass runs on the Scalar engine
            as Identity(src * rstd + (-mean * rstd)), halving the Vector load.
            """
            st = stat.tile([128, 2, 6], F32, name="st", tag="st")
            nc.vector.bn_stats(out=st[:, 0, :], in_=src_tile[:, 0:512])
            nc.vector.bn_stats(out=st[:, 1, :], in_=src_tile[:, 512:1024])
            mv = stat.tile([128, 2], F32, name="mv", tag="mv")
            nc.vector.bn_aggr(out=mv, in_=st)
            std = stat.tile([128, 1], F32, name="std", tag="std")
            nc.scalar.activation(
                out=std, in_=mv[:, 1:2], func=AF.Sqrt, bias=eps_t, scale=1.0
            )
            rstd = stat.tile([128, 1], F32, name="rstd", tag="rstd")
            nc.vector.reciprocal(out=rstd, in_=std)
            if use_scalar:
                nm = stat.tile([128, 1], F32, name="nm", tag="nm")
                nc.vector.tensor_scalar(
                    out=nm,
                    in0=mv[:, 0:1],
                    scalar1=rstd,
                    scalar2=-1.0,
                    op0=OP.mult,
                    op1=OP.mult,
                )
                nc.scalar.activation(
                    out=dst_tile, in_=src_tile, func=AF.Identity, bias=nm, scale=rstd
                )
            else:
                nc.vector.tensor_scalar(
                    out=dst_tile,
                    in0=src_tile,
                    scalar1=mv[:, 0:1],
                    scalar2=rstd,
                    op0=OP.subtract,
                    op1=OP.mult,
                )

        # ── Phase A+B: LN1 on own 512-token chunk, transpose to x^T ─────
        # emb tiles are persistent: they feed LN1 here and the attn residual
        # in phase H (loaded once).
        xtp = tc.alloc_tile_pool(name="xtp", bufs=1)
        wqp = tc.alloc_tile_pool(name="wqp", bufs=1)
        qdr = tc.alloc_tile_pool(name="qdr", bufs=1)
        ab_tp = tc.alloc_tile_pool(name="ab_tp", bufs=4, space="PSUM")
        xt_tiles = [xtp.tile([128, TC], BF16, name=f"xt{k}") for k in range(8)]
        et_tiles = []
        for i in range(NTT):
            et = embs.tile([128, D], F32, name="et", tag=f"et{i}", bufs=1)
            nc.sync.dma_start(out=et, in_=emb[ts(i, 128), :])
            et_tiles.append(et)
        # projection weights, k-row-block tiles, loaded in consumption order
        wt = {}
        for p, wsrc in enumerate((wq, wk, wv)):
            for k in range(8):
                w_ = wqp.tile([128, H * HS], BF16, name=f"w{p}_{k}")
                nc.sync.dma_start(out=w_, in_=wsrc[ts(k, 128), :])
                wt[(p, k)] = w_
        xn_tiles = []
        for i in range(NTT):
            xn = xln.tile([128, D], F32, name="xn", tag="xn", bufs=4)
            layer_norm(et_tiles[i], xn, use_scalar=(i % 2 == 1))
            xn_tiles.append(xn)
        # k-major transposes: xt[k] completes in arrival order so the QKV
        # k-outer loop below can start on xt[0] immediately
        for k in range(8):
            for i in range(NTT):
                ps = ab_tp.tile([128, 128], F32, name="abtp", tag="abtp")
                nc.tensor.transpose(ps, xn_tiles[i][:, ts(k, 128)], identity)
                if k % 2 == 0:
                    nc.vector.tensor_copy(out=xt_tiles[k][:, ts(i, 128)], in_=ps)
                else:
                    nc.scalar.copy(out=xt_tiles[k][:, ts(i, 128)], in_=ps)
        ab_tp.release()

        # ── Phase C: QKV for ALL heads on own tokens, k-outer ───────────
        # One projection at a time; its 8 output row-blocks accumulate in all
        # 8 PSUM banks while the k blocks stream in. Drained row-block j is
        # exactly the payload for AllToAll destination core j.
        qk_ps = tc.alloc_tile_pool(name="qk_ps", bufs=8, space="PSUM")
        for p in range(3):
            pj = [qk_ps.tile([128, TC], F32, name=f"pj{j}", tag="pj") for j in range(8)]
            for k in range(8):
                for j in range(8):
                    nc.tensor.matmul(
                        pj[j],
                        lhsT=wt[(p, k)][:, ts(j, 128)],
                        rhs=xt_tiles[k],
                        start=(k == 0),
                        stop=(k == 7),
                    )
            for j in range(8):
                dr = qdr.tile([128, TC], BF16, name="dr", tag="dr", bufs=6)
                if p == 2:
                    nc.scalar.activation(
                        out=dr, in_=pj[j], func=AF.Identity,
                        bias=bqs[:, 8 * p + j : 8 * p + j + 1], scale=1.0,
                    )
                else:
                    nc.vector.tensor_scalar_add(
                        out=dr, in0=pj[j], scalar1=bqs[:, 8 * p + j : 8 * p + j + 1]
                    )
                nc.sync.dma_start(out=cc_q_in[j, p, :, :], in_=dr)

        # ── Phase D: AllToAll q/k/v head-wise ───────────────────────────
        nc.gpsimd.collective_compute(
            "AllToAll", OP.bypass, replica_groups=rg,
            ins=[cc_q_in.opt()], outs=[cc_q_out.opt()],
        )
        qdr.release()
        wqp.release()
        xtp.release()
        qk_ps.release()

        # attention-lifetime pools (released after the second AllToAll)
        qkres = tc.alloc_tile_pool(name="qkres", bufs=1)
        vsbp = tc.alloc_tile_pool(name="vsbp", bufs=1)
        vtp = tc.alloc_tile_pool(name="vtp", bufs=3)
        otp = tc.alloc_tile_pool(name="otp", bufs=2)
        ptp = tc.alloc_tile_pool(name="ptp", bufs=6)
        attnc = tc.alloc_tile_pool(name="attnc", bufs=1)

        qT = qkres.tile([128, NT], BF16, name="qT")
        kT = qkres.tile([128, NT], BF16, name="kT")
        v_sb = vsbp.tile([128, NT // 128, HPC, 65], BF16, name="v_sb")
        ones_f = attnc.tile([128, 64], F32, name="ones_f")
        nc.vector.memset(ones_f, 1.0)
        # ones rows of V (the denominator trick) are constant: fill them once
        nc.vector.tensor_copy(
            out=v_sb[:, :, :, 64:65],
            in_=ones_f[:, 0:64].rearrange("p (a b c) -> p a b c", a=NT // 128, b=HPC),
        )

        # readback closures: per src chunk, q^T/k^T slices and the V^T tile
        # (V^T -> natural V transposes are woven into the attention weave)
        def make_readback_ops():
            ops = []
            vt_tiles = {}

            def rb(src):
                nc.sync.dma_start(
                    out=qT[:, ts(src, TC)], in_=cc_q_out[src, 0, :, :]
                )
                nc.sync.dma_start(
                    out=kT[:, ts(src, TC)], in_=cc_q_out[src, 1, :, :]
                )
                vt = vtp.tile([128, TC], BF16, name="vt", tag="vt", bufs=3)
                nc.sync.dma_start(out=vt, in_=cc_q_out[src, 2, :, :])
                vt_tiles[src] = vt

            def tr(src, t):
                st_ = 4 * src + t
                ps = v_ps.tile([128, 128], BF16, name="tpb", tag="tpb")
                nc.tensor.transpose(ps, vt_tiles[src][:, ts(t, 128)], identity_b)
                nc.vector.tensor_copy(
                    out=v_sb[:, st_, :, 0:64],
                    in_=ps.rearrange("p (h e) -> p h e", h=HPC),
                )

            for src in range(NCORES):
                ops.append(lambda src=src: rb(src))
                for t in range(NTT):
                    ops.append(lambda src=src, t=t: tr(src, t))
            return ops

        # ── Phase F: causal attention, software-pipelined across chunks ──
        # Per chunk: S^T matmuls stream through PSUM banks, exp evacuates to
        # bf16 SBUF; causal masking is an in-place affine_select on GpSimd.
        v_ps = tc.alloc_tile_pool(name="v_ps", bufs=2, space="PSUM")
        s_ps = tc.alloc_tile_pool(name="s_ps", bufs=2, space="PSUM")
        o_ps = tc.alloc_tile_pool(name="o_ps", bufs=1, space="PSUM")

        # W1 partial preload: DMA queues are idle during attention, so 5 MB of
        # the reads finish long before the AllToAll needs the wires.
        w1pa = tc.alloc_tile_pool(name="w1pa", bufs=1, side="right")
        w1sb = [w1pa.tile([128, FF], BF16, name=f"w1sb{k}") for k in range(5)]
        for k in range(5):
            nc.sync.dma_start(out=w1sb[k], in_=w1[ts(k, 128), :])

        def build_score_ops(gc, pts):
            """Closures, each emitting one scores pair: 2 mms + exp (+ mask).

            Two s-tiles share one 2-bank PSUM tile so a single Exp covers
            1024 columns (halves the Scalar per-instruction overhead).
            Causal column-skipping: a diag-d tile's first 128*d query
            columns never survive the mask, so every stage (scores, exp,
            mask, PV) is restricted to the columns the next stage reads.
            """
            b = gc // 4
            lc = gc % 4
            nst = 4 * lc + 4  # s-tiles (128 wide) within this batch
            t0g = gc * TC
            ops = []
            for sp in range(nst // 2):
                for h in range(HPC):
                    def op(sp=sp, h=h):
                        d0 = 2 * sp - 4 * lc
                        cl = 128 * d0 if d0 > 0 else 0
                        ps_ = s_ps.tile([128, 2, TC], F32, name="ps_", tag="ps_")
                        pt_ = ptp.tile(
                            [128, 2, TC], BF16, name="pt_", tag="pt_", bufs=29
                        )
                        for hf in range(2):
                            stl = 2 * sp + hf
                            sg = b * 16 + stl
                            diag = stl - 4 * lc
                            c0 = 128 * diag if diag > 0 else 0
                            # heads use PE row-groups 0-63 / 64-127
                            nc.tensor.matmul(
                                ps_[:, hf, ds(c0, TC - c0)],
                                lhsT=kT[ts(h, 64), ts(sg, 128)],
                                rhs=qT[ts(h, 64), ds(t0g + c0, TC - c0)],
                                start=True,
                                stop=True,
                            )
                        nc.scalar.activation(
                            out=pt_[:, :, ds(cl, TC - cl)],
                            in_=ps_[:, :, ds(cl, TC - cl)],
                            func=AF.Exp,
                            scale=SCALE,
                        )
                        if d0 + 1 >= 0:
                            # causal triangle of the diagonal pair, zeroed in
                            # place on GpSimd: keep where col >= 128*diag+row
                            nc.gpsimd.affine_select(
                                out=pt_[:, :, ds(cl, TC - cl)],
                                in_=pt_[:, :, ds(cl, TC - cl)],
                                pattern=[[-128, 2], [1, TC - cl]],
                                compare_op=OP.is_ge,
                                fill=0.0,
                                base=cl - 128 * d0,
                                channel_multiplier=-1,
                            )
                        for hf in range(2):
                            stl = 2 * sp + hf
                            diag = stl - 4 * lc
                            c0 = 128 * diag if diag > 0 else 0
                            pts[(stl, h)] = (pt_, hf, c0)
                    ops.append(op)
            return ops

        def build_pv_ops(gc, pts):
            """Closures: PV matmuls head-alternating, then drains + DMA."""
            b = gc // 4
            lc = gc % 4
            nst = 4 * lc + 4
            po = {}
            oT_h = [None]
            ops = []

            def mm(h, stl):
                if stl == 0:
                    po[h] = o_ps.tile([65, TC], F32, name=f"po{h}", tag=f"po{h}", bufs=1)
                sg = b * 16 + stl
                pt_, hf, c0 = pts[(stl, h)]
                nc.tensor.matmul(
                    po[h][:, ds(c0, TC - c0)],
                    lhsT=v_sb[:, sg, h, :],
                    rhs=pt_[:, hf, ds(c0, TC - c0)],
                    start=(stl == 0),
                    stop=(stl == nst - 1),
                    skip_group_check=(c0 > 0),
                )

            def finish(h):
                if oT_h[0] is None:
                    oT_h[0] = otp.tile([128, TC], BF16, name="oT", tag="oT")
                nc.vector.tensor_copy(out=oT_h[0][ts(h, 64), :], in_=po[h][0:64, :])
                oTd = otp.tile([1, TC], BF16, name="oTd", tag="oTd")
                nc.vector.tensor_copy(out=oTd, in_=po[h][64:65, :])
                nc.sync.dma_start(out=cc_a_in[gc, 128 + h : 129 + h, :], in_=oTd)
                if h == HPC - 1:
                    nc.sync.dma_start(out=cc_a_in[gc, 0:128, :], in_=oT_h[0])

            for stl in range(nst):
                for h in range(HPC):
                    ops.append(lambda h=h, stl=stl: mm(h, stl))
            for h in range(HPC):
                ops.append(lambda h=h: finish(h))
            return ops

        # Fine-grained interleave: scores pairs of chunk c+1 are woven between
        # the PV matmuls of chunk c, so the PE's instruction stream always has
        # 128-contraction matmuls in flight (keeps the HAM clock monitor at
        # full rate — a long stretch of 64-row scores reads as "idle" to it)
        # and the Scalar engine's exp latency is hidden. The q/k/v readback
        # DMAs + V transposes are woven the same way into the first chunks.
        pend = make_readback_ops()
        for gc in range(NCORES):
            pts = {}
            s_ops = build_score_ops(gc, pts)
            ns, npv = len(s_ops), len(pend)
            pi = 0
            for si in range(ns):
                # pending ops LEAD the weave: the readback DMAs that feed a
                # scores matmul must be emitted before it (emission order is
                # program order for dependency tracking)
                target = (npv * (si + 1)) // ns
                while pi < target:
                    pend[pi]()
                    pi += 1
                s_ops[si]()
            while pi < npv:
                pend[pi]()
                pi += 1
            pend = build_pv_ops(gc, pts)
        for op in pend:
            op()

        # ── Phase G: AllToAll -> unnormalized attn^T + denoms, own tokens ──
        nc.gpsimd.collective_compute(
            "AllToAll", OP.bypass, replica_groups=rg, ins=[cc_a_in.opt()], outs=[cc_a_out.opt()]
        )
        o_ps.release()
        s_ps.release()
        v_ps.release()
        attnc.release()
        ptp.release()
        otp.release()
        vtp.release()
        vsbp.release()
        qkres.release()

        # ── Phase H: normalize + attn residual + LN2, y -> y^T ──────────
        asbp = tc.alloc_tile_pool(name="asbp", bufs=4, side="right")
        h_tp = tc.alloc_tile_pool(name="h_tp", bufs=5, space="PSUM")
        x2p = tc.alloc_tile_pool(name="x2p", bufs=1)
        ytp = tc.alloc_tile_pool(name="ytp", bufs=1)
        w1pb = tc.alloc_tile_pool(name="w1pb", bufs=1)
        x2_tiles = [x2p.tile([128, D], F32, name=f"x2_{i}") for i in range(NTT)]
        yt_tiles = [ytp.tile([128, TC], BF16, name=f"yt{k}") for k in range(8)]
        asb_tiles = []
        # all 16 denominator rows in one tile: one transpose + one reciprocal
        # per token tile instead of eight
        dnm_all = asbp.tile([16, TC], BF16, name="dnm_all", bufs=1)
        for c in range(NCORES):
            asb = asbp.tile([128, TC], BF16, name="asb", tag="asb", bufs=8)
            nc.sync.dma_start(out=asb, in_=cc_a_out[c, 0:128, :])
            nc.sync.dma_start(
                out=dnm_all[2 * c : 2 * c + 2, :], in_=cc_a_out[c, 128:130, :]
            )
            asb_tiles.append(asb)
        # W1 row-blocks 5-7: queued behind the asb DMAs, so they drain right
        # after the AllToAll completes instead of contending with it.
        for k in range(5, 8):
            w1sb.append(w1pb.tile([128, FF], BF16, name=f"w1sb{k}"))
            nc.sync.dma_start(out=w1sb[k], in_=w1[ts(k, 128), :])
        # i-major: each x2 tile completes as early as possible so its LN2
        # (emitted right after) overlaps the remaining residual work
        yn_tiles = []
        for i in range(NTT):
            pd = h_tp.tile([128, 16], BF16, name="hpd", tag="hpd", bufs=2)
            nc.tensor.transpose(pd, dnm_all[:, ts(i, 128)], identity_b[0:16, 0:16])
            rcp = asbp.tile([128, 16], F32, name="rcp", tag="rcp", bufs=2)
            nc.vector.reciprocal(out=rcp, in_=pd)
            for c in range(NCORES):
                pn = h_tp.tile([128, 128], BF16, name="htp", tag="htp", bufs=4)
                nc.tensor.transpose(pn, asb_tiles[c][:, ts(i, 128)], identity_b)
                for h in range(HPC):
                    nc.vector.scalar_tensor_tensor(
                        out=x2_tiles[i][:, ds(128 * c + 64 * h, 64)],
                        in0=pn[:, ts(h, 64)],
                        scalar=rcp[:, 2 * c + h : 2 * c + h + 1],
                        in1=et_tiles[i][:, ds(128 * c + 64 * h, 64)],
                        op0=OP.mult,
                        op1=OP.add,
                    )
            # shares the xn tag: the phase-A xn tiles are dead by now, and
            # reusing their buffers keeps the SBUF budget under the limit
            yn = xln.tile([128, D], F32, name="yn", tag="xn", bufs=4)
            layer_norm(x2_tiles[i], yn, use_scalar=(i % 2 == 1))
            yn_tiles.append(yn)
        # k-major transposes: yt[0] (which gates the FFN's first matmul)
        # completes first instead of last
        for k in range(8):
            for i in range(NTT):
                ps = h_tp.tile([128, 128], F32, name="htp2", tag="htp2", bufs=2)
                nc.tensor.transpose(ps, yn_tiles[i][:, ts(k, 128)], identity)
                if k % 2 == 0:
                    nc.vector.tensor_copy(out=yt_tiles[k][:, ts(i, 128)], in_=ps)
                else:
                    nc.scalar.copy(out=yt_tiles[k][:, ts(i, 128)], in_=ps)
        asbp.release()
        h_tp.release()

        # ── Phase J: FFN up-projection, h^T = relu(W1^T y^T + b1) ───────
        htp = tc.alloc_tile_pool(name="htp", bufs=1)
        w2sp = tc.alloc_tile_pool(name="w2sp", bufs=6)
        outsp = tc.alloc_tile_pool(name="outs", bufs=1)
        h_ps = tc.alloc_tile_pool(name="h_ps", bufs=4, space="PSUM")
        ht_tiles = [htp.tile([128, TC], BF16, name=f"ht{j}") for j in range(FF // 128)]
        out_sb = [outsp.tile([128, D], F32, name=f"osb{i}") for i in range(NTT)]
        # fold the down-projection bias into the residual while Vector is idle
        # (LN2 has already consumed x2, so this is safe)
        for i in range(NTT):
            nc.vector.tensor_add(out=x2_tiles[i], in0=x2_tiles[i], in1=b2b)
        for jg in range(16):
            phs = [h_ps.tile([128, TC], F32, name=f"ph{jj}", tag="ph") for jj in range(2)]
            for k in range(8):
                for jj in range(2):
                    nc.tensor.matmul(
                        phs[jj],
                        lhsT=w1sb[k][:, ds(256 * jg + 128 * jj, 128)],
                        rhs=yt_tiles[k],
                        start=(k == 0),
                        stop=(k == 7),
                    )
            for jj in range(2):
                jt = 2 * jg + jj
                nc.scalar.activation(
                    out=ht_tiles[jt],
                    in_=phs[jj],
                    func=AF.Relu,
                    bias=b1s[:, jt : jt + 1],
                    scale=1.0,
                )
        h_ps.release()

        # ── Phase K: FFN down-projection, natural [token, D] accumulation ──
        # lhsT is an h^T chunk reused for both 512-wide halves of W2's rows;
        # each token tile owns a 2-bank PSUM accumulator, so the output needs
        # no final transposes — just one residual add per tile.
        f_ps = tc.alloc_tile_pool(name="f_ps", bufs=4, space="PSUM")
        pfs = [f_ps.tile([128, D], F32, name=f"pf{i}", tag="pf") for i in range(NTT)]
        for jt in range(FF // 128):
            w2t = w2sp.tile([128, D], BF16, name="w2t", tag="w2t")
            nc.sync.dma_start(out=w2t, in_=w2[ts(jt, 128), :])
            for i in range(NTT):
                for dh in range(2):
                    nc.tensor.matmul(
                        pfs[i][:, ts(dh, 512)],
                        lhsT=ht_tiles[jt][:, ts(i, 128)],
                        rhs=w2t[:, ts(dh, 512)],
                        start=(jt == 0),
                        stop=(jt == FF // 128 - 1),
                    )
        for i in range(NTT):
            # half-width adds + writes so the output DMA starts draining
            # while the second half is still being summed
            for dh in range(2):
                nc.vector.tensor_add(
                    out=out_sb[i][:, ts(dh, 512)],
                    in0=pfs[i][:, ts(dh, 512)],
                    in1=x2_tiles[i][:, ts(dh, 512)],
                )
                nc.sync.dma_start(
                    out=out[ts(i, 128), ds(512 * dh, 512)],
                    in_=out_sb[i][:, ts(dh, 512)],
                )

        f_ps.release()
        outsp.release()
        w2sp.release()
        htp.release()
        w1pb.release()
        ytp.release()
        x2p.release()
        xln.release()
        stat.release()
        w1pa.release()
        embs.release()
        dram.release()
        const.release()
    nc.finalize()
    return nc


_NC = None


def _get_nc():
    global _NC
    if _NC is None:
        _NC = build()
    return _NC


def make_in_maps(embds, Wq, Wk, Wv, ln1_g, ln1_b, ln2_g, ln2_b, W1, b1, W2, b2):
    embds = np.ascontiguousarray(np.asarray(embds, dtype=np.float32)).reshape(NT, D)
    Wq = np.asarray(Wq, dtype=np.float32)
    Wk = np.asarray(Wk, dtype=np.float32)
    Wv = np.asarray(Wv, dtype=np.float32)
    W1 = np.ascontiguousarray(np.asarray(W1, dtype=np.float32))
    W2 = np.ascontiguousarray(np.asarray(W2, dtype=np.float32))
    b1 = np.asarray(b1, dtype=np.float32)
    b2 = np.asarray(b2, dtype=np.float32)
    g1 = np.asarray(ln1_g, dtype=np.float32)
    bb1 = np.asarray(ln1_b, dtype=np.float32)
    g2 = np.asarray(ln2_g, dtype=np.float32)
    bb2 = np.asarray(ln2_b, dtype=np.float32)

    # Fold LN1 gain/bias into the QKV projections:
    #   q = (xn*g1 + b1) @ Wq = xn @ (g1[:,None]*Wq) + b1@Wq
    def _w_full(W):
        Wf = W * g1[None, :, None]  # [H, D, HS]
        return np.ascontiguousarray(
            Wf.transpose(1, 0, 2).reshape(D, H * HS)
        ).astype(ml_dtypes.bfloat16)

    bq = np.einsum("d,hde->he", bb1, Wq).reshape(H * HS)  # [1024]
    bk = np.einsum("d,hde->he", bb1, Wk).reshape(H * HS)
    bv = np.einsum("d,hde->he", bb1, Wv).reshape(H * HS)
    # [128, 24]: column p*8+j = bias rows for projection p, feature block j
    bqkv = np.ascontiguousarray(
        np.concatenate(
            [bq.reshape(8, 128).T, bk.reshape(8, 128).T, bv.reshape(8, 128).T],
            axis=1,
        ).astype(np.float32)
    )

    # Fold LN2 gain/bias into the FFN up-projection:
    #   h_pre = (yn*g2 + b2ln) @ W1 + b1 = yn @ (g2[:,None]*W1) + (b2ln@W1 + b1)
    W1f = (W1 * g2[:, None]).astype(ml_dtypes.bfloat16)
    b1f = b1 + bb2 @ W1
    W2b = W2.astype(ml_dtypes.bfloat16)
    b1r = np.ascontiguousarray(b1f.reshape(FF // 128, 128).T.astype(np.float32))

    wq_full = _w_full(Wq)
    wk_full = _w_full(Wk)
    wv_full = _w_full(Wv)

    in_maps = []
    for c in range(NCORES):
        in_maps.append(
            {
                "emb": np.ascontiguousarray(embds[c * TC : (c + 1) * TC]),
                "wq": wq_full,
                "wk": wk_full,
                "wv": wv_full,
                "bqkv": bqkv,
                "w1": W1f,
                "w2": W2b,
                "b1r": b1r,
                "b2f": np.ascontiguousarray(b2),
            }
        )
    return in_maps


def run(in_maps, trace=False, **kwargs):
    from concourse.bass_utils import run_bass_kernel_spmd

    nc = _get_nc()
    return run_bass_kernel_spmd(
        nc, in_maps, core_ids=list(range(NCORES)), trace=trace, **kwargs
    )


def kernel(**inputs):
    in_maps = make_in_maps(**inputs)
    res = run(in_maps, trace=False)
    outs = [res.results[c]["out"] for c in range(NCORES)]
    return np.concatenate(outs, axis=0).reshape(B, T, D)
